# Initial kernel scaffold
#
"""Quaternionic linear layer on 8 TRN2 NeuronCores.

out = x @ M + bias, where M (128x128) is the quaternion-structured
expansion of the tiny weight [32, 32, 4]. Data-parallel: x rows are
sharded across 8 cores; M / bias / identity are replicated.

Per core (32768 rows):
  - x is streamed in chunks of 2048 rows laid out as [128 partitions,
    16 rows each] so every partition reads one contiguous 8KB run.
  - Each 128-row tile is transposed on TensorE (via identity matmul),
    the transposed tile is copied PSUM->SBUF with an f32->bf16 cast on
    ScalarE, then a bf16 matmul against M accumulates into PSUM, and
    VectorE adds the (pre-broadcast) bias while copying PSUM->SBUF.
  - Output chunks stream back with the same contiguous-run layout.
"""

import numpy as np

import concourse.bass as bass
import concourse.mybir as mybir
import concourse.tile as tile
from concourse.bass_utils import run_bass_kernel_spmd

B = 262144
D = 128
N_CORES = 8
ROWS = B // N_CORES          # 32768 rows per core
C = 16                       # rows per partition per chunk
CHUNK = 128 * C              # 2048 rows per chunk
N_CHUNKS = ROWS // CHUNK     # 16
GROUP = 4                    # 128-row tiles per PSUM bank group
GROUPS_PER_CHUNK = C // GROUP

_GRAPH = None


def _build_graph():
    nc = bass.Bass()
    x = nc.declare_dram_parameter("x", [ROWS, D], mybir.dt.float32, isOutput=False)
    mat = nc.declare_dram_parameter("mat", [D, D], mybir.dt.bfloat16, isOutput=False)
    ident = nc.declare_dram_parameter("ident", [D, D], mybir.dt.float32, isOutput=False)
    biasbc = nc.declare_dram_parameter(
        "biasbc", [128, GROUP * D], mybir.dt.float32, isOutput=False
    )
    out = nc.declare_dram_parameter("out", [ROWS, D], mybir.dt.float32, isOutput=True)

    xv = x.rearrange("(c p j) f -> c p (j f)", c=N_CHUNKS, p=128, j=C)
    ov = out.rearrange("(c p j) f -> c p (j f)", c=N_CHUNKS, p=128, j=C)

    with tile.TileContext(nc) as tc:
        with (
            tc.tile_pool(name="const", bufs=1) as const_pool,
            tc.tile_pool(name="xin", bufs=2) as xin_pool,
            tc.tile_pool(name="xt", bufs=3) as xt_pool,
            tc.tile_pool(name="oout", bufs=2) as out_pool,
            tc.tile_pool(name="ps_t", bufs=3, space="PSUM") as pst_pool,
            tc.tile_pool(name="ps_o", bufs=3, space="PSUM") as pso_pool,
        ):
            mat_sb = const_pool.tile([D, D], mybir.dt.bfloat16)
            nc.sync.dma_start(out=mat_sb[:], in_=mat[:])
            id_sb = const_pool.tile([D, D], mybir.dt.float32)
            nc.sync.dma_start(out=id_sb[:], in_=ident[:])
            bias_sb = const_pool.tile([128, GROUP * D], mybir.dt.float32)
            nc.sync.dma_start(out=bias_sb[:], in_=biasbc[:])

            for c in range(N_CHUNKS):
                x_sb = xin_pool.tile([128, C * D], mybir.dt.float32)
                nc.sync.dma_start(out=x_sb[:], in_=xv[c])
                o_sb = out_pool.tile([128, C * D], mybir.dt.float32)
                for g in range(GROUPS_PER_CHUNK):
                    xt_ps = pst_pool.tile([D, GROUP * D], mybir.dt.float32)
                    for j in range(GROUP):
                        t = g * GROUP + j
                        nc.tensor.transpose(
                            xt_ps[:, j * D : (j + 1) * D],
                            x_sb[:, t * D : (t + 1) * D],
                            id_sb[:],
                        )
                    xt_sb = xt_pool.tile([D, GROUP * D], mybir.dt.bfloat16)
                    nc.scalar.activation(
                        out=xt_sb[:],
                        in_=xt_ps[:],
                        func=mybir.ActivationFunctionType.Copy,
                    )
                    o_ps = pso_pool.tile([128, GROUP * D], mybir.dt.float32)
                    for j in range(GROUP):
                        nc.tensor.matmul(
                            o_ps[:, j * D : (j + 1) * D],
                            xt_sb[:, j * D : (j + 1) * D],
                            mat_sb[:],
                            start=True,
                            stop=True,
                        )
                    nc.vector.tensor_tensor(
                        out=o_sb[:, g * GROUP * D : (g + 1) * GROUP * D],
                        in0=o_ps[:],
                        in1=bias_sb[:],
                        op=mybir.AluOpType.add,
                    )
                nc.sync.dma_start(out=ov[c], in_=o_sb[:])
    return nc


def _build_M(weight):
    w = np.asarray(weight, dtype=np.float32)
    wa, wi, wj, wk = w[..., 0], w[..., 1], w[..., 2], w[..., 3]  # each [o, n]
    Q = np.zeros((32, 4, 32, 4), dtype=np.float32)  # [n, ci, o, co]
    Q[:, 0, :, 0], Q[:, 1, :, 0], Q[:, 2, :, 0], Q[:, 3, :, 0] = wa.T, -wi.T, -wj.T, -wk.T
    Q[:, 0, :, 1], Q[:, 1, :, 1], Q[:, 2, :, 1], Q[:, 3, :, 1] = wi.T, wa.T, wk.T, -wj.T
    Q[:, 0, :, 2], Q[:, 1, :, 2], Q[:, 2, :, 2], Q[:, 3, :, 2] = wj.T, -wk.T, wa.T, wi.T
    Q[:, 0, :, 3], Q[:, 1, :, 3], Q[:, 2, :, 3], Q[:, 3, :, 3] = wk.T, wj.T, -wi.T, wa.T
    return Q.reshape(128, 128)


def run(x, weight, bias, trace=False, **spmd_kwargs):
    global _GRAPH
    if _GRAPH is None:
        _GRAPH = _build_graph()
    nc = _GRAPH

    bf16 = mybir.dt.np(mybir.dt.bfloat16)
    M = _build_M(weight).astype(bf16)
    ident = np.eye(D, dtype=np.float32)
    biasbc = np.tile(np.asarray(bias, dtype=np.float32), (128, GROUP))

    x = np.ascontiguousarray(np.asarray(x, dtype=np.float32))
    in_maps = []
    for i in range(N_CORES):
        in_maps.append(
            {
                "x": x[i * ROWS : (i + 1) * ROWS],
                "mat": M,
                "ident": ident,
                "biasbc": biasbc,
            }
        )
    res = run_bass_kernel_spmd(
        nc, in_maps, core_ids=list(range(N_CORES)), trace=trace, **spmd_kwargs
    )
    out = np.concatenate([r["out"] for r in res.results], axis=0)
    return out, res


def kernel(x, weight, bias):
    out, _ = run(x, weight, bias, trace=False)
    return out


# revision 7
# speedup vs baseline: 3.7592x; 3.7592x over previous
"""Quaternionic linear layer on 8 TRN2 NeuronCores.

out = x @ M + bias, where M (128x128) is the quaternion-structured
expansion of the tiny weight [32, 32, 4]. Data-parallel: x rows are
sharded across 8 cores; M / bias / identity are replicated.

Per core (32768 rows):
  - x is streamed in chunks of 2048 rows laid out as [128 partitions,
    16 rows each] so every partition reads one contiguous 8KB run.
  - Each 128-row tile is transposed on TensorE (via identity matmul),
    the transposed tile is copied PSUM->SBUF with an f32->bf16 cast on
    ScalarE, then a bf16 matmul against M accumulates into PSUM, and
    VectorE adds the (pre-broadcast) bias while copying PSUM->SBUF.
  - Output chunks stream back with the same contiguous-run layout.
"""

import numpy as np

import concourse.bass as bass
import concourse.bacc as bacc
import concourse.mybir as mybir
import concourse.tile as tile
from concourse.bass_utils import run_bass_kernel_spmd

B = 262144
D = 128
N_CORES = 8
ROWS = B // N_CORES          # 32768 rows per core
C = 16                       # rows per partition per chunk
CHUNK = 128 * C              # 2048 rows per chunk
N_CHUNKS = ROWS // CHUNK     # 16
GROUP = 4                    # 128-row tiles per PSUM bank group
GROUPS_PER_CHUNK = C // GROUP

_GRAPH = None


def _build_graph(reps=1):
    nc = bacc.Bacc(None)
    x = nc.declare_dram_parameter("x", [ROWS, D], mybir.dt.float32, isOutput=False)
    mat = nc.declare_dram_parameter("mat", [D, D], mybir.dt.bfloat16, isOutput=False)
    ident = nc.declare_dram_parameter("ident", [D, D], mybir.dt.float32, isOutput=False)
    biasbc = nc.declare_dram_parameter(
        "biasbc", [128, GROUP * D], mybir.dt.float32, isOutput=False
    )
    out = nc.declare_dram_parameter("out", [ROWS, D], mybir.dt.float32, isOutput=True)

    xv = x.rearrange("(c p j) f -> c p (j f)", c=N_CHUNKS, p=128, j=C)
    ov = out.rearrange("(c p j) f -> c p (j f)", c=N_CHUNKS, p=128, j=C)

    bf16 = mybir.dt.bfloat16
    with tile.TileContext(nc) as tc:
        with (
            tc.tile_pool(name="const", bufs=1) as const_pool,
            tc.tile_pool(name="xin", bufs=2) as xin_pool,
            tc.tile_pool(name="xt", bufs=3) as xt_pool,
            tc.tile_pool(name="oout", bufs=2) as out_pool,
            tc.tile_pool(name="ps_t", bufs=3, space="PSUM") as pst_pool,
            tc.tile_pool(name="ps_o", bufs=3, space="PSUM") as pso_pool,
        ):
            mat_sb = const_pool.tile([D, D], bf16)
            nc.sync.dma_start(out=mat_sb[:], in_=mat[:])
            id_sb = const_pool.tile([D, D], mybir.dt.float32)
            nc.sync.dma_start(out=id_sb[:], in_=ident[:])
            bias_sb = const_pool.tile([128, GROUP * D], mybir.dt.float32)
            nc.sync.dma_start(out=bias_sb[:], in_=biasbc[:])

            for c in range(N_CHUNKS * reps):
                c = c % N_CHUNKS
                x_sb = xin_pool.tile([128, C * D], mybir.dt.float32)
                nc.sync.dma_start(out=x_sb[:], in_=xv[c])
                o_sb = out_pool.tile([128, C * D], mybir.dt.float32)
                for g in range(GROUPS_PER_CHUNK):
                    xt_ps = pst_pool.tile([D, GROUP * D], mybir.dt.float32)
                    for j in range(GROUP):
                        t = g * GROUP + j
                        nc.tensor.transpose(
                            xt_ps[:, j * D : (j + 1) * D],
                            x_sb[:, t * D : (t + 1) * D],
                            id_sb[:],
                        )
                    xt_sb = xt_pool.tile([D, GROUP * D], bf16)
                    nc.scalar.activation(
                        out=xt_sb[:],
                        in_=xt_ps[:],
                        func=mybir.ActivationFunctionType.Copy,
                    )
                    o_ps = pso_pool.tile([128, GROUP * D], mybir.dt.float32)
                    for j in range(GROUP):
                        nc.tensor.matmul(
                            o_ps[:, j * D : (j + 1) * D],
                            xt_sb[:, j * D : (j + 1) * D],
                            mat_sb[:],
                            start=True,
                            stop=True,
                        )
                    nc.vector.tensor_tensor(
                        out=o_sb[:, g * GROUP * D : (g + 1) * GROUP * D],
                        in0=o_ps[:],
                        in1=bias_sb[:],
                        op=mybir.AluOpType.add,
                    )
                nc.sync.dma_start(out=ov[c], in_=o_sb[:])
    nc.finalize()
    return nc


def _build_M(weight):
    w = np.asarray(weight, dtype=np.float32)
    wa, wi, wj, wk = w[..., 0], w[..., 1], w[..., 2], w[..., 3]  # each [o, n]
    Q = np.zeros((32, 4, 32, 4), dtype=np.float32)  # [n, ci, o, co]
    Q[:, 0, :, 0], Q[:, 1, :, 0], Q[:, 2, :, 0], Q[:, 3, :, 0] = wa.T, -wi.T, -wj.T, -wk.T
    Q[:, 0, :, 1], Q[:, 1, :, 1], Q[:, 2, :, 1], Q[:, 3, :, 1] = wi.T, wa.T, wk.T, -wj.T
    Q[:, 0, :, 2], Q[:, 1, :, 2], Q[:, 2, :, 2], Q[:, 3, :, 2] = wj.T, -wk.T, wa.T, wi.T
    Q[:, 0, :, 3], Q[:, 1, :, 3], Q[:, 2, :, 3], Q[:, 3, :, 3] = wk.T, wj.T, -wi.T, wa.T
    return Q.reshape(128, 128)


def run(x, weight, bias, trace=False, **spmd_kwargs):
    global _GRAPH
    if _GRAPH is None:
        _GRAPH = _build_graph()
    nc = _GRAPH

    bf16 = mybir.dt.np(mybir.dt.bfloat16)
    M = _build_M(weight).astype(bf16)
    ident = np.eye(D, dtype=np.float32)
    biasbc = np.tile(np.asarray(bias, dtype=np.float32), (128, GROUP))

    x = np.ascontiguousarray(np.asarray(x, dtype=np.float32))
    in_maps = []
    for i in range(N_CORES):
        in_maps.append(
            {
                "x": x[i * ROWS : (i + 1) * ROWS],
                "mat": M,
                "ident": ident,
                "biasbc": biasbc,
            }
        )
    res = run_bass_kernel_spmd(
        nc, in_maps, core_ids=list(range(N_CORES)), trace=trace, **spmd_kwargs
    )
    out = np.concatenate([r["out"] for r in res.results], axis=0)
    return out, res


def kernel(x, weight, bias):
    out, _ = run(x, weight, bias, trace=False)
    return out
